# revision 43
# baseline (speedup 1.0000x reference)
"""Multi-head attention (B=4, N=2048, C=1024, H=16) on 8 TRN2 NeuronCores. v3.

Sharding: (batch, head-group) pairs -> 8 cores. Core c handles batch c//2 and
heads [(c%2)*8, (c%2)*8+8). QKV weights column-sharded per head group, proj
row-sharded; each core emits a partial proj output (transposed); host sums the
two partials per batch and adds b_proj.

v2 (402.7us) -> v6 (391.9us). Main findings along the way:
  - Filler MMs (qkv-gen/proj) are interleaved 1-3 per kc instead of v2's
    4-8 MM bursts every 4th kc, at generator granularity.
  - Unit epilogues (PSUM evacuation on ACT/DVE) are deferred and flushed
    at the top of the next kc-batch: emitted right after their last MM
    they head-of-line block the strict-FIFO ACT/DVE queues (their wait
    spans the spread-out MM group), delaying the exps whose completion
    releases the ss PSUM banks, which showed up as ~105ns of wait inside
    the score MMs. Flushing must happen before the batch's score MMs:
    dependency tracking is emission-ordered, so a consumer emitted before
    the writer's epilogue reads garbage (race, not an error).
  - Scores are emitted as two serial K=64 MMs (row-tile concurrency
    still happens via base_partition auto-derivation when the scheduler
    lines them up); explicit tile_position batching variants and a
    shared-LDWEIGHTS variant (InstMatmult.ldweights=False) all measured
    slower; 1024-wide 2-bank exp batching starved the ss pipeline
    (bufs=1) and also lost.
  - DMA prologue: xT DMA'd in 4 token-block chunks per row chunk, wqk
    host-side column-permuted into consumer priority order
    (k4,q0,k5,q1,k6,q2,k7,q3), k4's block DMA'd first.
  - v tiles store 65 cols/head (64 v + 1 ones, no pad).
  - norm split: z-copy+reciprocal+GPSIMD broadcast at the pair boundary,
    the ao muls a batch later (keeps the DVE queue from blocking behind
    the GPSIMD broadcast). reciprocal_approx_fast must NOT read PSUM
    directly (garbage on HW - bitcast reads of PSUM don't work).
  - tail: j=3 proj units run 3 of their 4 accumulation MMs mid-loop,
    DMA'd as partial output yT2 that the host adds; 8 single MMs remain
    after the final norm, kept warm by dummy MMs over the norm chain.
  - Schraudolph exp on DVE handles kc<14, ACT takes kc 14-15 (exact).
  - Beware: the chassis intermittently throttles the PE to 2.0GHz (P0
    power state); such runs measure ~480us - check MM min durations
    (215ns = 2.4GHz, 258ns = 2.0GHz) before comparing.
"""

import os
import sys

import numpy as np

for _p in ("/root/.axon_site", "/root/.axon_site/_ro/trn_rl_repo",
           "/root/.axon_site/_ro/pypackages", "/opt/trn_rl_repo", "/opt/pypackages"):
    if os.path.isdir(_p) and _p not in sys.path:
        sys.path.append(_p)

import concourse.bacc as bacc
import concourse.mybir as mybir
import concourse.tile as tile
from concourse.bass_utils import run_bass_kernel_spmd

B, N, C = 4, 2048, 1024
H, D = 16, 64
NH = 8            # heads per core
CL = NH * D       # 512 local channels
NCORES = 8
SCALE = float(D) ** -0.5

F32 = mybir.dt.float32
BF16 = mybir.dt.bfloat16
I16 = mybir.dt.int16

MM_DT = "bf16"

# Schraudolph bf16 exp: bits16 = round(s * 128/ln2 + (127*128 - 7.5))
A_EXP = 184.66509399414062
B_EXP = 16248.5

# wqk column-block priority order: position -> logical m (m 0..3 = q pair m,
# m 4..7 = k pair m-4). First consumers first: k4 (prologue), q0, k5, q1, ...
PR = [4, 0, 5, 1, 6, 2, 7, 3]
POS = [PR.index(m) for m in range(8)]   # logical m -> column block position

_CACHE = {}


def build_nc():
    nc = bacc.Bacc()

    xT = nc.declare_dram_parameter("xT", [C, N], BF16, isOutput=False)
    wqk = nc.declare_dram_parameter("wqk", [C, 2 * CL], BF16, isOutput=False)
    wv = nc.declare_dram_parameter("wv", [C, CL], BF16, isOutput=False)
    wp = nc.declare_dram_parameter("wp", [CL, C], BF16, isOutput=False)
    bqk = nc.declare_dram_parameter("bqk", [128, 8], F32, isOutput=False)
    yT = nc.declare_dram_parameter("yT", [C, N], F32, isOutput=True)
    # second partial for the j=3 token block (tail split); host adds
    yT2 = nc.declare_dram_parameter("yT2", [C, 512], F32, isOutput=True)

    Ident = mybir.ActivationFunctionType.Identity
    Exp = mybir.ActivationFunctionType.Exp
    Copy = mybir.ActivationFunctionType.Copy
    Mult = mybir.AluOpType.mult
    Add = mybir.AluOpType.add

    with tile.TileContext(nc) as tc:
        with (
            tc.tile_pool(name="const", bufs=1) as const,
            tc.tile_pool(name="wpool", bufs=1) as wpool,
            tc.tile_pool(name="qkpool", bufs=1) as qkpool,
            tc.tile_pool(name="vpool", bufs=1) as vpool,
            tc.tile_pool(name="aopool", bufs=1) as aopool,
            tc.tile_pool(name="xpool", bufs=1) as xpool,
            tc.tile_pool(name="ptpool", bufs=16) as ptpool,
            tc.tile_pool(name="rpool", bufs=3) as rpool,
            tc.tile_pool(name="ytpool", bufs=4) as ytpool,
        ):
            bqk_t = const.tile([128, 8], F32, tag="bqk", name="bqk")
            nc.sync.dma_start(out=bqk_t[:], in_=bqk[:])

            # ---- tiles (DMA'd in priority order below) ----
            xT_t, wqk_t, wv_t, wp_t = [], [], [], []
            for cc in range(8):
                xT_t.append(xpool.tile([128, N], BF16, tag=f"xT{cc}", name=f"xT{cc}"))
                wqk_t.append(wpool.tile([128, 2 * CL], BF16, tag=f"wqk{cc}",
                                        name=f"wqk{cc}"))
                wv_t.append(wpool.tile([128, CL], BF16, tag=f"wv{cc}", name=f"wv{cc}"))
            for cl in range(4):
                wp_t.append(wpool.tile([128, C], BF16, tag=f"wp{cl}", name=f"wp{cl}"))

            def dma_wqk_half(h):
                cs = slice(h * 512, (h + 1) * 512)
                for cc in range(8):
                    nc.sync.dma_start(out=wqk_t[cc][:, cs],
                                      in_=wqk[cc * 128:(cc + 1) * 128, cs])

            def dma_xT(j):
                js = slice(j * 512, (j + 1) * 512)
                for cc in range(8):
                    nc.sync.dma_start(out=xT_t[cc][:, js],
                                      in_=xT[cc * 128:(cc + 1) * 128, js])

            # Priority order: interleave k4's column block with xT j0 per
            # C-chunk — the first k-gen MM (cc=0) needs only the cc=0 pair
            # (~160 KiB), and each subsequent MM's data lands just ahead of
            # it. Then the rest of wqk's first half (q0, k5, q1).
            for cc in range(8):
                nc.sync.dma_start(out=wqk_t[cc][:, 0:128],
                                  in_=wqk[cc * 128:(cc + 1) * 128, 0:128])
                nc.sync.dma_start(out=xT_t[cc][:, 0:512],
                                  in_=xT[cc * 128:(cc + 1) * 128, 0:512])
            for cc in range(8):
                nc.sync.dma_start(out=wqk_t[cc][:, 128:512],
                                  in_=wqk[cc * 128:(cc + 1) * 128, 128:512])
            for cc in range(8):
                nc.sync.dma_start(out=wv_t[cc][:], in_=wv[cc * 128:(cc + 1) * 128, :])
            dma_xT(1)
            dma_wqk_half(1)
            dma_xT(2)
            dma_xT(3)
            for cl in range(4):
                nc.sync.dma_start(out=wp_t[cl][:], in_=wp[cl * 128:(cl + 1) * 128, :])

            # persistent intermediates
            qk_t = []   # m 0..3 -> q^T chunks (scaled+biased), 4..7 -> k^T
            for m in range(8):
                qk_t.append(qkpool.tile([128, N], BF16, tag=f"qk{m}", name=f"qk{m}"))
            v_t = []    # [v | 1] per head: 8 groups of 65 cols
            for kc in range(16):
                v_t.append(vpool.tile([128, NH * 65], BF16, tag=f"v{kc}",
                                      name=f"v{kc}"))
            ao_t = []   # ao_t[p]: pair p normalized out^T (128 ch x N)
            for p in range(4):
                ao_t.append(aopool.tile([128, N], BF16, tag=f"ao{p}", name=f"ao{p}"))

            # ---------------- unit generators (one PE MM per step) --------
            # All unit PSUM goes through `aux` (allocated lazily at first
            # step so buffer rotation follows true issue order).
            # Unit epilogues (PSUM evacuation on ACT/DVE) are NOT emitted
            # with the final MM: they go into epi_q and are flushed at the
            # next iteration top, AFTER that iteration's exps are emitted.
            # Otherwise the epilogue sits in the strict-FIFO ACT/DVE queue
            # waiting for its spread-out MM group and head-of-line blocks
            # the exps, which stalls the PE on ss-bank reuse.
            aux_ref = [None]
            epi_q = []

            def flush_epi():
                while epi_q:
                    epi_q.pop(0)()

            def qk_gen(m, j):
                aux = aux_ref[0]
                js = slice(j * 512, (j + 1) * 512)
                ws = slice(POS[m] * 128, (POS[m] + 1) * 128)
                ps = aux.tile([128, 512], F32, tag="ps", name="ps")
                for cc in range(8):
                    nc.tensor.matmul(ps[:], wqk_t[cc][:, ws], xT_t[cc][:, js],
                                     start=(cc == 0), stop=(cc == 7))
                    if cc < 7:
                        yield

                def epi():
                    if (m + j) % 2 == 1:
                        nc.vector.tensor_scalar(
                            qk_t[m][:, js], ps[:], SCALE if m < 4 else 1.0,
                            bqk_t[:, m:m + 1], Mult, Add)
                    else:
                        nc.scalar.activation(
                            qk_t[m][:, js], ps[:], Ident,
                            bias=bqk_t[:, m:m + 1],
                            scale=SCALE if m < 4 else 1.0)
                epi_q.append(epi)
                yield

            def v_gen(kc):
                aux = aux_ref[0]
                ps = aux.tile([128, 512], F32, tag="ps", name="ps")
                for cc in range(8):
                    nc.tensor.matmul(ps[:], xT_t[cc][:, kc * 128:(kc + 1) * 128],
                                     wv_t[cc][:], start=(cc == 0), stop=(cc == 7))
                    if cc < 7:
                        yield

                def epi():
                    v3 = v_t[kc].rearrange("p (h e) -> p h e", h=NH)
                    nc.vector.memset(v3[:, :, 64:65], 1.0)
                    nc.vector.tensor_copy(
                        v3[:, :, 0:64],
                        ps.rearrange("p (h e) -> p h e", e=64))
                epi_q.append(epi)
                yield

            def proj_gen(m2, j, cls=(0, 1, 2, 3), out_dram=None, pool=None,
                         tag="ps"):
                aux = pool if pool is not None else aux_ref[0]
                js = slice(j * 512, (j + 1) * 512)
                py = aux.tile([128, 512], F32, tag=tag, name="py")
                for i, cl in enumerate(cls):
                    nc.tensor.matmul(py[:], wp_t[cl][:, m2 * 128:(m2 + 1) * 128],
                                     ao_t[cl][:, js], start=(i == 0),
                                     stop=(i == len(cls) - 1))
                    if i < len(cls) - 1:
                        yield

                def epi():
                    yt = ytpool.tile([128, 512], F32, tag="yt", name="yt")
                    # in-loop: ACT (DVE is the more loaded engine mid-loop);
                    # tail (yT2): alternate so the 8 copies split engines
                    if out_dram is not None and m2 % 2 == 0:
                        nc.vector.tensor_copy(yt[:], py[:])
                    else:
                        nc.scalar.activation(yt[:], py[:], Copy)
                    if out_dram is None:
                        nc.sync.dma_start(out=yT[m2 * 128:(m2 + 1) * 128, js],
                                          in_=yt[:])
                    else:
                        nc.sync.dma_start(out=out_dram[m2 * 128:(m2 + 1) * 128, :],
                                          in_=yt[:])
                epi_q.append(epi)
                yield

            # ---------------- filler machinery ----------------
            queue = []          # pending generators
            cur = [None]        # in-flight generator

            def push(*gens):
                queue.extend(gens)

            def fill(n):
                for _ in range(n):
                    while True:
                        if cur[0] is None:
                            if not queue:
                                return
                            cur[0] = queue.pop(0)
                        try:
                            next(cur[0])
                            break
                        except StopIteration:
                            cur[0] = None

            def run_unit(g):
                for _ in g:
                    pass

            # ---------------- attention helpers ----------------
            def sc_pair(ssEp, ssOp, p, kc, js):
                # serial full-mode score MMs (no tile_position): row-tile
                # pairing saved ~216ns/kc of overlap but cost ~105ns leader
                # wait + ~119ns tile<->full mode-switch penalty on the next
                # full-mode MM - a net loss.
                ssE = ssEp.tile([128, 512], F32, tag="ssE", name="ssE")
                ssO = ssOp.tile([128, 512], F32, tag="ssO", name="ssO")
                ks = slice(kc * 128, (kc + 1) * 128)
                nc.tensor.matmul(ssE[:], qk_t[4 + p][0:64, ks], qk_t[p][0:64, js],
                                 start=True, stop=True)
                nc.tensor.matmul(ssO[:], qk_t[4 + p][64:128, ks],
                                 qk_t[p][64:128, js], start=True, stop=True)
                return ssE, ssO

            def exp_pair(ssE, ssO, kc):
                ptE = ptpool.tile([128, 512], BF16, tag="ptE", name="ptE")
                ptO = ptpool.tile([128, 512], BF16, tag="ptO", name="ptO")
                nc.scalar.activation(ptE[:], ssE[:], Exp)
                if kc < 14:
                    nc.vector.tensor_scalar(
                        ptO.bitcast(I16)[:], ssO[:], A_EXP, B_EXP, Mult, Add)
                else:
                    nc.scalar.activation(ptO[:], ssO[:], Exp)
                return ptE, ptO

            def av_pair(avE, avO, p, kc, pts):
                ptE, ptO = pts[kc]
                he, ho = 2 * p, 2 * p + 1
                nc.tensor.matmul(avE[0:65, :], v_t[kc][:, he * 65:he * 65 + 65],
                                 ptE[:], start=(kc == 0), stop=(kc == 15))
                nc.tensor.matmul(avO[0:65, :], v_t[kc][:, ho * 65:ho * 65 + 65],
                                 ptO[:], start=(kc == 0), stop=(kc == 15))

            def norm_pre(av):
                # av rows 0:64 = out^T, row 64 = Z. Split from the mul so
                # the DVE mul (which waits on the GPSIMD broadcast) can be
                # emitted a batch later and never blocks the DVE queue.
                z1 = rpool.tile([1, 512], F32, tag="z1", name="z1")
                nc.vector.tensor_copy(z1[:], av[64:65, :])
                r1 = rpool.tile([1, 512], F32, tag="r1", name="r1")
                nc.vector.reciprocal_approx_fast(out=r1[:], in_=z1[:])
                rb = rpool.tile([64, 512], F32, tag="rb", name="rb")
                nc.gpsimd.partition_broadcast(rb[:], r1[:])
                return rb

            def norm_mul(av, p, po, js_, rb):
                nc.vector.tensor_mul(ao_t[p][po:po + 64, js_], av[0:64, :], rb[:])

            # ---------------- main ----------------
            with (
                tc.tile_pool(name="ssE", bufs=2, space="PSUM") as ssEp,
                tc.tile_pool(name="ssO", bufs=2, space="PSUM") as ssOp,
                tc.tile_pool(name="avE", bufs=1, space="PSUM") as avEp,
                tc.tile_pool(name="avO", bufs=1, space="PSUM") as avOp,
                tc.tile_pool(name="aux", bufs=2, space="PSUM") as aux,
            ):
                aux_ref[0] = aux

                # prologue: k(m4) j0/j1, q(m0) j0, v(0..3); k(m4) j2/j3 via queue
                for g in (qk_gen(4, 0), qk_gen(0, 0), v_gen(0), v_gen(1),
                          v_gen(2), v_gen(3), qk_gen(4, 1)):
                    run_unit(g)
                    flush_epi()

                push(qk_gen(4, 2), qk_gen(4, 3),
                     qk_gen(1, 0), qk_gen(5, 0), qk_gen(5, 1), qk_gen(5, 2),
                     qk_gen(5, 3), qk_gen(2, 0), qk_gen(6, 0), qk_gen(6, 1),
                     qk_gen(6, 2), qk_gen(6, 3), qk_gen(3, 0), qk_gen(7, 0),
                     qk_gen(7, 1), qk_gen(7, 2), qk_gen(7, 3),
                     qk_gen(0, 1), qk_gen(1, 1), qk_gen(2, 1), qk_gen(3, 1))

                pend = [None]
                for ji in range(4):
                    for p in range(4):
                        j = ji
                        js = slice(j * 512, (j + 1) * 512)
                        if (ji, p) == (1, 0):
                            push(*[proj_gen(m2, 0) for m2 in range(8)])
                            push(qk_gen(0, 2), qk_gen(1, 2), qk_gen(2, 2),
                                 qk_gen(3, 2))
                        elif (ji, p) == (2, 0):
                            push(*[proj_gen(m2, 1) for m2 in range(8)])
                            push(qk_gen(0, 3), qk_gen(1, 3), qk_gen(2, 3),
                                 qk_gen(3, 3))
                        elif (ji, p) == (3, 0):
                            push(*[proj_gen(m2, 2) for m2 in range(8)])

                        # kc processed in batches of 2: both score pairs run
                        # back-to-back in row-tile mode (one tile<->full mode
                        # switch per batch instead of per kc; each switch
                        # costs the first following MM ~105-120ns).
                        pts = {}
                        avE = avO = None
                        for kc2 in range(8):
                            a, b = 2 * kc2, 2 * kc2 + 1
                            # flush BEFORE the ss MMs: emission order defines
                            # dependency tracking, so any epilogue whose
                            # output the ss (or other) MMs read must be
                            # emitted first. Units always finish >=1 fill
                            # slot before the flush, so the epilogue's wait
                            # is (near-)resolved and doesn't HOL-block exps.
                            flush_epi()
                            ssA = sc_pair(ssEp, ssOp, p, a, js)
                            pts[a] = exp_pair(*ssA, a)
                            ssB = sc_pair(ssEp, ssOp, p, b, js)
                            pts[b] = exp_pair(*ssB, b)
                            if pend[0] is not None and kc2 == 0:
                                pavE, pavO, pp, pjs, ppts = pend[0]
                                av_pair(pavE, pavO, pp, 14, ppts)
                                av_pair(pavE, pavO, pp, 15, ppts)
                                rbE = norm_pre(pavE)
                                rbO = norm_pre(pavO)
                                pend[0] = (pavE, pavO, pp, pjs, rbE, rbO)
                            if kc2 == 1:
                                if pend[0] is not None:
                                    pavE, pavO, pp, pjs, rbE, rbO = pend[0]
                                    norm_mul(pavE, pp, 0, pjs, rbE)
                                    norm_mul(pavO, pp, 64, pjs, rbO)
                                    pend[0] = None
                                avE = avEp.tile([128, 512], F32, tag="avE",
                                                name="avE")
                                avO = avOp.tile([128, 512], F32, tag="avO",
                                                name="avO")
                            if kc2 >= 1:
                                av_pair(avE, avO, p, a - 2, pts)
                                av_pair(avE, avO, p, b - 2, pts)
                            # fillers: spread evenly, heavier early (qkv gen)
                            if (ji, p) == (0, 0):
                                if kc2 < 6:
                                    run_unit(v_gen(a + 4))
                                    run_unit(v_gen(b + 4))
                                    fill(4)
                                else:
                                    fill(6)
                            elif ji == 0:
                                fill(6)
                            elif (ji, p) == (3, 3):
                                if kc2 == 1:
                                    push(*[proj_gen(m2, 3, cls=(0, 1, 2))
                                           for m2 in range(8)])
                                fill(4)
                            else:
                                fill(2)
                        pend[0] = (avE, avO, p, js, pts)

                # ---------------- tail ----------------
                pavE, pavO, pp, pjs, ppts = pend[0]
                av_pair(pavE, pavO, pp, 14, ppts)
                av_pair(pavE, pavO, pp, 15, ppts)
                rbE = norm_pre(pavE)
                rbO = norm_pre(pavO)
                norm_mul(pavE, pp, 0, pjs, rbE)
                norm_mul(pavO, pp, 64, pjs, rbO)
                pend[0] = None
                fill(64)   # drain any queue leftovers
                flush_epi()
                # keep-warm dummies while the final norm chain runs
                dum = aux.tile([128, 512], F32, tag="ps", name="dum")
                for _ in range(22):
                    nc.tensor.matmul(dum[:], wp_t[0][:, 0:128], ao_t[0][:, 0:512],
                                     start=True, stop=True)
                # final 8 single-MM proj units (cl=3 only) -> yT2. Cycle
                # their PSUM through the now-free av banks too (4-deep
                # rotation): with aux's 2 buffers alone, unit m2+2's MM
                # stalls on unit m2's epilogue copy.
                tail_pools = [(avEp, "avE"), (avOp, "avO"), (aux, "ps"),
                              (aux, "ps")]
                for m2 in range(8):
                    tp_, tg_ = tail_pools[m2 % 4]
                    run_unit(proj_gen(m2, 3, cls=(3,), out_dram=yT2,
                                      pool=tp_, tag=tg_))
                    flush_epi()

    nc.compile()
    return nc


def make_in_maps(x, w_qkv, b_qkv, w_proj):
    np_bf = mybir.dt.np(BF16)
    x = np.asarray(x, np.float32)
    w_qkv = np.asarray(w_qkv, np.float32)
    b_qkv = np.asarray(b_qkv, np.float32)
    w_proj = np.asarray(w_proj, np.float32)
    in_maps = []
    for c in range(NCORES):
        b, g = divmod(c, 2)
        h0 = g * NH
        qs = slice(h0 * D, h0 * D + CL)
        ks = slice(C + h0 * D, C + h0 * D + CL)
        vs = slice(2 * C + h0 * D, 2 * C + h0 * D + CL)
        wqk_m = np.concatenate([w_qkv[:, qs], w_qkv[:, ks]], axis=1)
        # permute 128-col blocks into consumer priority order
        wqk_m = np.concatenate([wqk_m[:, m * 128:(m + 1) * 128] for m in PR],
                               axis=1)
        bq = b_qkv[qs] * SCALE
        bk = b_qkv[ks]
        bqk_m = np.concatenate([bq, bk]).reshape(8, 128).T  # [128, 8] col-chunks
        in_maps.append({
            "xT": np.ascontiguousarray(x[b].T).astype(np_bf),
            "wqk": np.ascontiguousarray(wqk_m).astype(np_bf),
            "wv": np.ascontiguousarray(w_qkv[:, vs]).astype(np_bf),
            "wp": np.ascontiguousarray(w_proj[h0 * D:h0 * D + CL, :]).astype(np_bf),
            "bqk": np.ascontiguousarray(bqk_m, np.float32),
        })
    return in_maps


def run(x, w_qkv, b_qkv, w_proj, b_proj, mm_dt=MM_DT, **spmd_kwargs):
    if "nc" not in _CACHE:
        _CACHE["nc"] = build_nc()
    nc = _CACHE["nc"]
    in_maps = make_in_maps(x, w_qkv, b_qkv, w_proj)
    res = run_bass_kernel_spmd(nc, in_maps, core_ids=list(range(NCORES)),
                               **spmd_kwargs)
    # v-bias passes through softmax averaging exactly (weights sum to 1),
    # so its projected contribution folds into the output bias on the host.
    b_eff = (np.asarray(b_proj, np.float64)
             + np.asarray(b_qkv, np.float64)[2 * C:] @ np.asarray(w_proj, np.float64)
             ).astype(np.float32)
    out = np.empty((B, N, C), np.float32)
    for b in range(B):
        acc = res.results[2 * b]["yT"] + res.results[2 * b + 1]["yT"]
        acc[:, 3 * 512:] += res.results[2 * b]["yT2"] + res.results[2 * b + 1]["yT2"]
        out[b] = acc.T + b_eff[None, :]
    return out, res


def kernel(x, w_qkv, b_qkv, w_proj, b_proj):
    out, _ = run(x, w_qkv, b_qkv, w_proj, b_proj)
    return out


# revision 44
# speedup vs baseline: 1.0038x; 1.0038x over previous
"""Multi-head attention (B=4, N=2048, C=1024, H=16) on 8 TRN2 NeuronCores. v3.

Sharding: (batch, head-group) pairs -> 8 cores. Core c handles batch c//2 and
heads [(c%2)*8, (c%2)*8+8). QKV weights column-sharded per head group, proj
row-sharded; each core emits a partial proj output (transposed); host sums the
two partials per batch and adds b_proj.

v2 (402.7us) -> v6 (391.9us). Main findings along the way:
  - Filler MMs (qkv-gen/proj) are interleaved 1-3 per kc instead of v2's
    4-8 MM bursts every 4th kc, at generator granularity.
  - Unit epilogues (PSUM evacuation on ACT/DVE) are deferred and flushed
    at the top of the next kc-batch: emitted right after their last MM
    they head-of-line block the strict-FIFO ACT/DVE queues (their wait
    spans the spread-out MM group), delaying the exps whose completion
    releases the ss PSUM banks, which showed up as ~105ns of wait inside
    the score MMs. Flushing must happen before the batch's score MMs:
    dependency tracking is emission-ordered, so a consumer emitted before
    the writer's epilogue reads garbage (race, not an error).
  - Scores are emitted as two serial K=64 MMs (row-tile concurrency
    still happens via base_partition auto-derivation when the scheduler
    lines them up); explicit tile_position batching variants and a
    shared-LDWEIGHTS variant (InstMatmult.ldweights=False) all measured
    slower; 1024-wide 2-bank exp batching starved the ss pipeline
    (bufs=1) and also lost.
  - DMA prologue: xT DMA'd in 4 token-block chunks per row chunk, wqk
    host-side column-permuted into consumer priority order
    (k4,q0,k5,q1,k6,q2,k7,q3), k4's block DMA'd first.
  - v tiles store 65 cols/head (64 v + 1 ones, no pad).
  - norm split: z-copy+reciprocal+GPSIMD broadcast at the pair boundary,
    the ao muls a batch later (keeps the DVE queue from blocking behind
    the GPSIMD broadcast). reciprocal_approx_fast must NOT read PSUM
    directly (garbage on HW - bitcast reads of PSUM don't work).
  - tail: j=3 proj units run 3 of their 4 accumulation MMs mid-loop,
    DMA'd as partial output yT2 that the host adds; 8 single MMs remain
    after the final norm, kept warm by dummy MMs over the norm chain.
  - Schraudolph exp on DVE handles kc<14, ACT takes kc 14-15 (exact).
  - Beware: the chassis intermittently throttles the PE to 2.0GHz (P0
    power state); such runs measure ~480us - check MM min durations
    (215ns = 2.4GHz, 258ns = 2.0GHz) before comparing.
"""

import os
import sys

import numpy as np

for _p in ("/root/.axon_site", "/root/.axon_site/_ro/trn_rl_repo",
           "/root/.axon_site/_ro/pypackages", "/opt/trn_rl_repo", "/opt/pypackages"):
    if os.path.isdir(_p) and _p not in sys.path:
        sys.path.append(_p)

import concourse.bacc as bacc
import concourse.mybir as mybir
import concourse.tile as tile
from concourse.bass_utils import run_bass_kernel_spmd

B, N, C = 4, 2048, 1024
H, D = 16, 64
NH = 8            # heads per core
CL = NH * D       # 512 local channels
NCORES = 8
SCALE = float(D) ** -0.5

F32 = mybir.dt.float32
BF16 = mybir.dt.bfloat16
I16 = mybir.dt.int16

MM_DT = "bf16"

# Schraudolph bf16 exp: bits16 = round(s * 128/ln2 + (127*128 - 7.5))
A_EXP = 184.66509399414062
B_EXP = 16248.5

# wqk column-block priority order: position -> logical m (m 0..3 = q pair m,
# m 4..7 = k pair m-4). First consumers first: k4 (prologue), q0, k5, q1, ...
PR = [4, 0, 5, 1, 6, 2, 7, 3]
POS = [PR.index(m) for m in range(8)]   # logical m -> column block position

_CACHE = {}


def build_nc():
    nc = bacc.Bacc()

    xT = nc.declare_dram_parameter("xT", [C, N], BF16, isOutput=False)
    wqk = nc.declare_dram_parameter("wqk", [C, 2 * CL], BF16, isOutput=False)
    wv = nc.declare_dram_parameter("wv", [C, CL], BF16, isOutput=False)
    wp = nc.declare_dram_parameter("wp", [CL, C], BF16, isOutput=False)
    bqk = nc.declare_dram_parameter("bqk", [128, 8], F32, isOutput=False)
    yT = nc.declare_dram_parameter("yT", [C, N], F32, isOutput=True)
    # second partial for the j=3 token block (tail split); host adds
    yT2 = nc.declare_dram_parameter("yT2", [C, 512], F32, isOutput=True)

    Ident = mybir.ActivationFunctionType.Identity
    Exp = mybir.ActivationFunctionType.Exp
    Copy = mybir.ActivationFunctionType.Copy
    Mult = mybir.AluOpType.mult
    Add = mybir.AluOpType.add

    with tile.TileContext(nc) as tc:
        with (
            tc.tile_pool(name="const", bufs=1) as const,
            tc.tile_pool(name="wpool", bufs=1) as wpool,
            tc.tile_pool(name="qkpool", bufs=1) as qkpool,
            tc.tile_pool(name="vpool", bufs=1) as vpool,
            tc.tile_pool(name="aopool", bufs=1) as aopool,
            tc.tile_pool(name="xpool", bufs=1) as xpool,
            tc.tile_pool(name="ptpool", bufs=16) as ptpool,
            tc.tile_pool(name="rpool", bufs=3) as rpool,
            tc.tile_pool(name="ytpool", bufs=4) as ytpool,
        ):
            bqk_t = const.tile([128, 8], F32, tag="bqk", name="bqk")
            nc.sync.dma_start(out=bqk_t[:], in_=bqk[:])

            # ---- tiles (DMA'd in priority order below) ----
            xT_t, wqk_t, wv_t, wp_t = [], [], [], []
            for cc in range(8):
                xT_t.append(xpool.tile([128, N], BF16, tag=f"xT{cc}", name=f"xT{cc}"))
                wqk_t.append(wpool.tile([128, 2 * CL], BF16, tag=f"wqk{cc}",
                                        name=f"wqk{cc}"))
                wv_t.append(wpool.tile([128, CL], BF16, tag=f"wv{cc}", name=f"wv{cc}"))
            for cl in range(4):
                wp_t.append(wpool.tile([128, C], BF16, tag=f"wp{cl}", name=f"wp{cl}"))

            def dma_wqk_half(h):
                cs = slice(h * 512, (h + 1) * 512)
                for cc in range(8):
                    nc.sync.dma_start(out=wqk_t[cc][:, cs],
                                      in_=wqk[cc * 128:(cc + 1) * 128, cs])

            def dma_xT(j):
                js = slice(j * 512, (j + 1) * 512)
                for cc in range(8):
                    nc.sync.dma_start(out=xT_t[cc][:, js],
                                      in_=xT[cc * 128:(cc + 1) * 128, js])

            # Priority order: k4's column block first (small lines but only
            # 256 KiB — lets the first k-gen unit start ~2us earlier), then
            # xT j0, then the rest of wqk's first half (q0, k5, q1).
            for cc in range(8):
                nc.sync.dma_start(out=wqk_t[cc][:, 0:128],
                                  in_=wqk[cc * 128:(cc + 1) * 128, 0:128])
            dma_xT(0)
            for cc in range(8):
                nc.sync.dma_start(out=wqk_t[cc][:, 128:512],
                                  in_=wqk[cc * 128:(cc + 1) * 128, 128:512])
            for cc in range(8):
                nc.sync.dma_start(out=wv_t[cc][:], in_=wv[cc * 128:(cc + 1) * 128, :])
            dma_xT(1)
            dma_wqk_half(1)
            dma_xT(2)
            dma_xT(3)
            for cl in range(4):
                nc.sync.dma_start(out=wp_t[cl][:], in_=wp[cl * 128:(cl + 1) * 128, :])

            # persistent intermediates
            qk_t = []   # m 0..3 -> q^T chunks (scaled+biased), 4..7 -> k^T
            for m in range(8):
                qk_t.append(qkpool.tile([128, N], BF16, tag=f"qk{m}", name=f"qk{m}"))
            v_t = []    # [v | 1] per head: 8 groups of 65 cols
            for kc in range(16):
                v_t.append(vpool.tile([128, NH * 65], BF16, tag=f"v{kc}",
                                      name=f"v{kc}"))
            ao_t = []   # ao_t[p]: pair p normalized out^T (128 ch x N)
            for p in range(4):
                ao_t.append(aopool.tile([128, N], BF16, tag=f"ao{p}", name=f"ao{p}"))

            # ---------------- unit generators (one PE MM per step) --------
            # All unit PSUM goes through `aux` (allocated lazily at first
            # step so buffer rotation follows true issue order).
            # Unit epilogues (PSUM evacuation on ACT/DVE) are NOT emitted
            # with the final MM: they go into epi_q and are flushed at the
            # next iteration top, AFTER that iteration's exps are emitted.
            # Otherwise the epilogue sits in the strict-FIFO ACT/DVE queue
            # waiting for its spread-out MM group and head-of-line blocks
            # the exps, which stalls the PE on ss-bank reuse.
            aux_ref = [None]
            epi_q = []

            def flush_epi():
                while epi_q:
                    epi_q.pop(0)()

            def qk_gen(m, j):
                aux = aux_ref[0]
                js = slice(j * 512, (j + 1) * 512)
                ws = slice(POS[m] * 128, (POS[m] + 1) * 128)
                ps = aux.tile([128, 512], F32, tag="ps", name="ps")
                for cc in range(8):
                    nc.tensor.matmul(ps[:], wqk_t[cc][:, ws], xT_t[cc][:, js],
                                     start=(cc == 0), stop=(cc == 7))
                    if cc < 7:
                        yield

                def epi():
                    if (m + j) % 2 == 1:
                        nc.vector.tensor_scalar(
                            qk_t[m][:, js], ps[:], SCALE if m < 4 else 1.0,
                            bqk_t[:, m:m + 1], Mult, Add)
                    else:
                        nc.scalar.activation(
                            qk_t[m][:, js], ps[:], Ident,
                            bias=bqk_t[:, m:m + 1],
                            scale=SCALE if m < 4 else 1.0)
                epi_q.append(epi)
                yield

            def v_gen(kc):
                aux = aux_ref[0]
                ps = aux.tile([128, 512], F32, tag="ps", name="ps")
                for cc in range(8):
                    nc.tensor.matmul(ps[:], xT_t[cc][:, kc * 128:(kc + 1) * 128],
                                     wv_t[cc][:], start=(cc == 0), stop=(cc == 7))
                    if cc < 7:
                        yield

                def epi():
                    v3 = v_t[kc].rearrange("p (h e) -> p h e", h=NH)
                    nc.vector.memset(v3[:, :, 64:65], 1.0)
                    nc.vector.tensor_copy(
                        v3[:, :, 0:64],
                        ps.rearrange("p (h e) -> p h e", e=64))
                epi_q.append(epi)
                yield

            def proj_gen(m2, j, cls=(0, 1, 2, 3), out_dram=None):
                aux = aux_ref[0]
                js = slice(j * 512, (j + 1) * 512)
                py = aux.tile([128, 512], F32, tag="ps", name="py")
                for i, cl in enumerate(cls):
                    nc.tensor.matmul(py[:], wp_t[cl][:, m2 * 128:(m2 + 1) * 128],
                                     ao_t[cl][:, js], start=(i == 0),
                                     stop=(i == len(cls) - 1))
                    if i < len(cls) - 1:
                        yield

                def epi():
                    yt = ytpool.tile([128, 512], F32, tag="yt", name="yt")
                    # in-loop: ACT (DVE is the more loaded engine mid-loop);
                    # tail (yT2): alternate so the 8 copies split engines
                    if out_dram is not None and m2 % 2 == 0:
                        nc.vector.tensor_copy(yt[:], py[:])
                    else:
                        nc.scalar.activation(yt[:], py[:], Copy)
                    if out_dram is None:
                        nc.sync.dma_start(out=yT[m2 * 128:(m2 + 1) * 128, js],
                                          in_=yt[:])
                    else:
                        nc.sync.dma_start(out=out_dram[m2 * 128:(m2 + 1) * 128, :],
                                          in_=yt[:])
                epi_q.append(epi)
                yield

            # ---------------- filler machinery ----------------
            queue = []          # pending generators
            cur = [None]        # in-flight generator

            def push(*gens):
                queue.extend(gens)

            def fill(n):
                for _ in range(n):
                    while True:
                        if cur[0] is None:
                            if not queue:
                                return
                            cur[0] = queue.pop(0)
                        try:
                            next(cur[0])
                            break
                        except StopIteration:
                            cur[0] = None

            def run_unit(g):
                for _ in g:
                    pass

            # ---------------- attention helpers ----------------
            def sc_pair(ssEp, ssOp, p, kc, js):
                # serial full-mode score MMs (no tile_position): row-tile
                # pairing saved ~216ns/kc of overlap but cost ~105ns leader
                # wait + ~119ns tile<->full mode-switch penalty on the next
                # full-mode MM - a net loss.
                ssE = ssEp.tile([128, 512], F32, tag="ssE", name="ssE")
                ssO = ssOp.tile([128, 512], F32, tag="ssO", name="ssO")
                ks = slice(kc * 128, (kc + 1) * 128)
                nc.tensor.matmul(ssE[:], qk_t[4 + p][0:64, ks], qk_t[p][0:64, js],
                                 start=True, stop=True)
                nc.tensor.matmul(ssO[:], qk_t[4 + p][64:128, ks],
                                 qk_t[p][64:128, js], start=True, stop=True)
                return ssE, ssO

            def exp_pair(ssE, ssO, kc):
                ptE = ptpool.tile([128, 512], BF16, tag="ptE", name="ptE")
                ptO = ptpool.tile([128, 512], BF16, tag="ptO", name="ptO")
                nc.scalar.activation(ptE[:], ssE[:], Exp)
                if kc < 14:
                    nc.vector.tensor_scalar(
                        ptO.bitcast(I16)[:], ssO[:], A_EXP, B_EXP, Mult, Add)
                else:
                    nc.scalar.activation(ptO[:], ssO[:], Exp)
                return ptE, ptO

            def av_pair(avE, avO, p, kc, pts):
                ptE, ptO = pts[kc]
                he, ho = 2 * p, 2 * p + 1
                nc.tensor.matmul(avE[0:65, :], v_t[kc][:, he * 65:he * 65 + 65],
                                 ptE[:], start=(kc == 0), stop=(kc == 15))
                nc.tensor.matmul(avO[0:65, :], v_t[kc][:, ho * 65:ho * 65 + 65],
                                 ptO[:], start=(kc == 0), stop=(kc == 15))

            def norm_pre(av):
                # av rows 0:64 = out^T, row 64 = Z. Split from the mul so
                # the DVE mul (which waits on the GPSIMD broadcast) can be
                # emitted a batch later and never blocks the DVE queue.
                z1 = rpool.tile([1, 512], F32, tag="z1", name="z1")
                nc.vector.tensor_copy(z1[:], av[64:65, :])
                r1 = rpool.tile([1, 512], F32, tag="r1", name="r1")
                nc.vector.reciprocal_approx_fast(out=r1[:], in_=z1[:])
                rb = rpool.tile([64, 512], F32, tag="rb", name="rb")
                nc.gpsimd.partition_broadcast(rb[:], r1[:])
                return rb

            def norm_mul(av, p, po, js_, rb):
                nc.vector.tensor_mul(ao_t[p][po:po + 64, js_], av[0:64, :], rb[:])

            # ---------------- main ----------------
            with (
                tc.tile_pool(name="ssE", bufs=2, space="PSUM") as ssEp,
                tc.tile_pool(name="ssO", bufs=2, space="PSUM") as ssOp,
                tc.tile_pool(name="avE", bufs=1, space="PSUM") as avEp,
                tc.tile_pool(name="avO", bufs=1, space="PSUM") as avOp,
                tc.tile_pool(name="aux", bufs=2, space="PSUM") as aux,
            ):
                aux_ref[0] = aux

                # prologue: k(m4) j0/j1, q(m0) j0, v(0..3); k(m4) j2/j3 via queue
                for g in (qk_gen(4, 0), qk_gen(0, 0), v_gen(0), v_gen(1),
                          v_gen(2), v_gen(3), qk_gen(4, 1)):
                    run_unit(g)
                    flush_epi()

                push(qk_gen(4, 2), qk_gen(4, 3),
                     qk_gen(1, 0), qk_gen(5, 0), qk_gen(5, 1), qk_gen(5, 2),
                     qk_gen(5, 3), qk_gen(2, 0), qk_gen(6, 0), qk_gen(6, 1),
                     qk_gen(6, 2), qk_gen(6, 3), qk_gen(3, 0), qk_gen(7, 0),
                     qk_gen(7, 1), qk_gen(7, 2), qk_gen(7, 3),
                     qk_gen(0, 1), qk_gen(1, 1), qk_gen(2, 1), qk_gen(3, 1))

                pend = [None]
                for ji in range(4):
                    for p in range(4):
                        j = ji
                        js = slice(j * 512, (j + 1) * 512)
                        if (ji, p) == (1, 0):
                            push(*[proj_gen(m2, 0) for m2 in range(8)])
                            push(qk_gen(0, 2), qk_gen(1, 2), qk_gen(2, 2),
                                 qk_gen(3, 2))
                        elif (ji, p) == (2, 0):
                            push(*[proj_gen(m2, 1) for m2 in range(8)])
                            push(qk_gen(0, 3), qk_gen(1, 3), qk_gen(2, 3),
                                 qk_gen(3, 3))
                        elif (ji, p) == (3, 0):
                            push(*[proj_gen(m2, 2) for m2 in range(8)])

                        # kc processed in batches of 2: both score pairs run
                        # back-to-back in row-tile mode (one tile<->full mode
                        # switch per batch instead of per kc; each switch
                        # costs the first following MM ~105-120ns).
                        pts = {}
                        avE = avO = None
                        for kc2 in range(8):
                            a, b = 2 * kc2, 2 * kc2 + 1
                            # flush BEFORE the ss MMs: emission order defines
                            # dependency tracking, so any epilogue whose
                            # output the ss (or other) MMs read must be
                            # emitted first. Units always finish >=1 fill
                            # slot before the flush, so the epilogue's wait
                            # is (near-)resolved and doesn't HOL-block exps.
                            flush_epi()
                            ssA = sc_pair(ssEp, ssOp, p, a, js)
                            pts[a] = exp_pair(*ssA, a)
                            ssB = sc_pair(ssEp, ssOp, p, b, js)
                            pts[b] = exp_pair(*ssB, b)
                            if pend[0] is not None and kc2 == 0:
                                pavE, pavO, pp, pjs, ppts = pend[0]
                                av_pair(pavE, pavO, pp, 14, ppts)
                                av_pair(pavE, pavO, pp, 15, ppts)
                                rbE = norm_pre(pavE)
                                rbO = norm_pre(pavO)
                                pend[0] = (pavE, pavO, pp, pjs, rbE, rbO)
                            if kc2 == 1:
                                if pend[0] is not None:
                                    pavE, pavO, pp, pjs, rbE, rbO = pend[0]
                                    norm_mul(pavE, pp, 0, pjs, rbE)
                                    norm_mul(pavO, pp, 64, pjs, rbO)
                                    pend[0] = None
                                avE = avEp.tile([128, 512], F32, tag="avE",
                                                name="avE")
                                avO = avOp.tile([128, 512], F32, tag="avO",
                                                name="avO")
                            if kc2 >= 1:
                                av_pair(avE, avO, p, a - 2, pts)
                                av_pair(avE, avO, p, b - 2, pts)
                            # fillers: spread evenly, heavier early (qkv gen)
                            if (ji, p) == (0, 0):
                                if kc2 < 6:
                                    run_unit(v_gen(a + 4))
                                    run_unit(v_gen(b + 4))
                                    fill(4)
                                else:
                                    fill(6)
                            elif ji == 0:
                                fill(6)
                            elif (ji, p) == (3, 3):
                                if kc2 == 1:
                                    push(*[proj_gen(m2, 3, cls=(0, 1, 2))
                                           for m2 in range(8)])
                                fill(4)
                            else:
                                fill(2)
                        pend[0] = (avE, avO, p, js, pts)

                # ---------------- tail ----------------
                pavE, pavO, pp, pjs, ppts = pend[0]
                av_pair(pavE, pavO, pp, 14, ppts)
                av_pair(pavE, pavO, pp, 15, ppts)
                rbE = norm_pre(pavE)
                rbO = norm_pre(pavO)
                norm_mul(pavE, pp, 0, pjs, rbE)
                norm_mul(pavO, pp, 64, pjs, rbO)
                pend[0] = None
                fill(64)   # drain any queue leftovers
                flush_epi()
                # keep-warm dummies while the final norm chain runs
                dum = aux.tile([128, 512], F32, tag="ps", name="dum")
                for _ in range(22):
                    nc.tensor.matmul(dum[:], wp_t[0][:, 0:128], ao_t[0][:, 0:512],
                                     start=True, stop=True)
                # final 8 single-MM proj units (cl=3 only) -> yT2
                for m2 in range(8):
                    run_unit(proj_gen(m2, 3, cls=(3,), out_dram=yT2))
                    flush_epi()

    nc.compile()
    return nc


def make_in_maps(x, w_qkv, b_qkv, w_proj):
    np_bf = mybir.dt.np(BF16)
    x = np.asarray(x, np.float32)
    w_qkv = np.asarray(w_qkv, np.float32)
    b_qkv = np.asarray(b_qkv, np.float32)
    w_proj = np.asarray(w_proj, np.float32)
    in_maps = []
    for c in range(NCORES):
        b, g = divmod(c, 2)
        h0 = g * NH
        qs = slice(h0 * D, h0 * D + CL)
        ks = slice(C + h0 * D, C + h0 * D + CL)
        vs = slice(2 * C + h0 * D, 2 * C + h0 * D + CL)
        wqk_m = np.concatenate([w_qkv[:, qs], w_qkv[:, ks]], axis=1)
        # permute 128-col blocks into consumer priority order
        wqk_m = np.concatenate([wqk_m[:, m * 128:(m + 1) * 128] for m in PR],
                               axis=1)
        bq = b_qkv[qs] * SCALE
        bk = b_qkv[ks]
        bqk_m = np.concatenate([bq, bk]).reshape(8, 128).T  # [128, 8] col-chunks
        in_maps.append({
            "xT": np.ascontiguousarray(x[b].T).astype(np_bf),
            "wqk": np.ascontiguousarray(wqk_m).astype(np_bf),
            "wv": np.ascontiguousarray(w_qkv[:, vs]).astype(np_bf),
            "wp": np.ascontiguousarray(w_proj[h0 * D:h0 * D + CL, :]).astype(np_bf),
            "bqk": np.ascontiguousarray(bqk_m, np.float32),
        })
    return in_maps


def run(x, w_qkv, b_qkv, w_proj, b_proj, mm_dt=MM_DT, **spmd_kwargs):
    if "nc" not in _CACHE:
        _CACHE["nc"] = build_nc()
    nc = _CACHE["nc"]
    in_maps = make_in_maps(x, w_qkv, b_qkv, w_proj)
    res = run_bass_kernel_spmd(nc, in_maps, core_ids=list(range(NCORES)),
                               **spmd_kwargs)
    # v-bias passes through softmax averaging exactly (weights sum to 1),
    # so its projected contribution folds into the output bias on the host.
    b_eff = (np.asarray(b_proj, np.float64)
             + np.asarray(b_qkv, np.float64)[2 * C:] @ np.asarray(w_proj, np.float64)
             ).astype(np.float32)
    out = np.empty((B, N, C), np.float32)
    for b in range(B):
        acc = res.results[2 * b]["yT"] + res.results[2 * b + 1]["yT"]
        acc[:, 3 * 512:] += res.results[2 * b]["yT2"] + res.results[2 * b + 1]["yT2"]
        out[b] = acc.T + b_eff[None, :]
    return out, res


def kernel(x, w_qkv, b_qkv, w_proj, b_proj):
    out, _ = run(x, w_qkv, b_qkv, w_proj, b_proj)
    return out


# revision 46
# speedup vs baseline: 1.0050x; 1.0011x over previous
"""Multi-head attention (B=4, N=2048, C=1024, H=16) on 8 TRN2 NeuronCores. v3.

Sharding: (batch, head-group) pairs -> 8 cores. Core c handles batch c//2 and
heads [(c%2)*8, (c%2)*8+8). QKV weights column-sharded per head group, proj
row-sharded; each core emits a partial proj output (transposed); host sums the
two partials per batch and adds b_proj.

v2 (402.7us) -> v6 (391.9us). Main findings along the way:
  - Filler MMs (qkv-gen/proj) are interleaved 1-3 per kc instead of v2's
    4-8 MM bursts every 4th kc, at generator granularity.
  - Unit epilogues (PSUM evacuation on ACT/DVE) are deferred and flushed
    at the top of the next kc-batch: emitted right after their last MM
    they head-of-line block the strict-FIFO ACT/DVE queues (their wait
    spans the spread-out MM group), delaying the exps whose completion
    releases the ss PSUM banks, which showed up as ~105ns of wait inside
    the score MMs. Flushing must happen before the batch's score MMs:
    dependency tracking is emission-ordered, so a consumer emitted before
    the writer's epilogue reads garbage (race, not an error).
  - Scores are emitted as two serial K=64 MMs (row-tile concurrency
    still happens via base_partition auto-derivation when the scheduler
    lines them up); explicit tile_position batching variants and a
    shared-LDWEIGHTS variant (InstMatmult.ldweights=False) all measured
    slower; 1024-wide 2-bank exp batching starved the ss pipeline
    (bufs=1) and also lost.
  - DMA prologue: xT DMA'd in 4 token-block chunks per row chunk, wqk
    host-side column-permuted into consumer priority order
    (k4,q0,k5,q1,k6,q2,k7,q3), k4's block DMA'd first.
  - v tiles store 65 cols/head (64 v + 1 ones, no pad).
  - norm split: z-copy+reciprocal+GPSIMD broadcast at the pair boundary,
    the ao muls a batch later (keeps the DVE queue from blocking behind
    the GPSIMD broadcast). reciprocal_approx_fast must NOT read PSUM
    directly (garbage on HW - bitcast reads of PSUM don't work).
  - tail: j=3 proj units run 3 of their 4 accumulation MMs mid-loop,
    DMA'd as partial output yT2 that the host adds; 8 single MMs remain
    after the final norm, kept warm by dummy MMs over the norm chain.
  - Schraudolph exp on DVE handles kc<14, ACT takes kc 14-15 (exact).
  - Beware: the chassis intermittently throttles the PE to 2.0GHz (P0
    power state); such runs measure ~480us - check MM min durations
    (215ns = 2.4GHz, 258ns = 2.0GHz) before comparing.
"""

import os
import sys

import numpy as np

for _p in ("/root/.axon_site", "/root/.axon_site/_ro/trn_rl_repo",
           "/root/.axon_site/_ro/pypackages", "/opt/trn_rl_repo", "/opt/pypackages"):
    if os.path.isdir(_p) and _p not in sys.path:
        sys.path.append(_p)

import concourse.bacc as bacc
import concourse.mybir as mybir
import concourse.tile as tile
from concourse.bass_utils import run_bass_kernel_spmd

B, N, C = 4, 2048, 1024
H, D = 16, 64
NH = 8            # heads per core
CL = NH * D       # 512 local channels
NCORES = 8
SCALE = float(D) ** -0.5

F32 = mybir.dt.float32
BF16 = mybir.dt.bfloat16
I16 = mybir.dt.int16

MM_DT = "bf16"

# Schraudolph bf16 exp: bits16 = round(s * 128/ln2 + (127*128 - 7.5))
A_EXP = 184.66509399414062
B_EXP = 16248.5

# wqk column-block priority order: position -> logical m (m 0..3 = q pair m,
# m 4..7 = k pair m-4). First consumers first: k4 (prologue), q0, k5, q1, ...
PR = [4, 0, 5, 1, 6, 2, 7, 3]
POS = [PR.index(m) for m in range(8)]   # logical m -> column block position

_CACHE = {}


def build_nc():
    nc = bacc.Bacc()

    xT = nc.declare_dram_parameter("xT", [C, N], BF16, isOutput=False)
    wqk = nc.declare_dram_parameter("wqk", [C, 2 * CL], BF16, isOutput=False)
    wv = nc.declare_dram_parameter("wv", [C, CL], BF16, isOutput=False)
    wp = nc.declare_dram_parameter("wp", [CL, C], BF16, isOutput=False)
    bqk = nc.declare_dram_parameter("bqk", [128, 8], F32, isOutput=False)
    yT = nc.declare_dram_parameter("yT", [C, N], F32, isOutput=True)
    # second partial for the j=3 token block (tail split); host adds
    yT2 = nc.declare_dram_parameter("yT2", [C, 512], F32, isOutput=True)

    Ident = mybir.ActivationFunctionType.Identity
    Exp = mybir.ActivationFunctionType.Exp
    Copy = mybir.ActivationFunctionType.Copy
    Mult = mybir.AluOpType.mult
    Add = mybir.AluOpType.add

    with tile.TileContext(nc) as tc:
        with (
            tc.tile_pool(name="const", bufs=1) as const,
            tc.tile_pool(name="wpool", bufs=1) as wpool,
            tc.tile_pool(name="qkpool", bufs=1) as qkpool,
            tc.tile_pool(name="vpool", bufs=1) as vpool,
            tc.tile_pool(name="aopool", bufs=1) as aopool,
            tc.tile_pool(name="xpool", bufs=1) as xpool,
            tc.tile_pool(name="ptpool", bufs=16) as ptpool,
            tc.tile_pool(name="rpool", bufs=3) as rpool,
            tc.tile_pool(name="ytpool", bufs=4) as ytpool,
        ):
            bqk_t = const.tile([128, 8], F32, tag="bqk", name="bqk")
            nc.sync.dma_start(out=bqk_t[:], in_=bqk[:])

            # ---- tiles (DMA'd in priority order below) ----
            xT_t, wqk_t, wv_t, wp_t = [], [], [], []
            for cc in range(8):
                xT_t.append(xpool.tile([128, N], BF16, tag=f"xT{cc}", name=f"xT{cc}"))
                wqk_t.append(wpool.tile([128, 2 * CL], BF16, tag=f"wqk{cc}",
                                        name=f"wqk{cc}"))
                wv_t.append(wpool.tile([128, CL], BF16, tag=f"wv{cc}", name=f"wv{cc}"))
            for cl in range(4):
                wp_t.append(wpool.tile([128, C], BF16, tag=f"wp{cl}", name=f"wp{cl}"))

            def dma_wqk_half(h):
                cs = slice(h * 512, (h + 1) * 512)
                for cc in range(8):
                    nc.sync.dma_start(out=wqk_t[cc][:, cs],
                                      in_=wqk[cc * 128:(cc + 1) * 128, cs])

            def dma_xT(j):
                js = slice(j * 512, (j + 1) * 512)
                for cc in range(8):
                    nc.sync.dma_start(out=xT_t[cc][:, js],
                                      in_=xT[cc * 128:(cc + 1) * 128, js])

            # Priority order: k4's column block first (small lines but only
            # 256 KiB — lets the first k-gen unit start ~2us earlier), then
            # xT j0, then the rest of wqk's first half (q0, k5, q1).
            for cc in range(8):
                nc.sync.dma_start(out=wqk_t[cc][:, 0:128],
                                  in_=wqk[cc * 128:(cc + 1) * 128, 0:128])
            dma_xT(0)
            for cc in range(8):
                nc.sync.dma_start(out=wqk_t[cc][:, 128:512],
                                  in_=wqk[cc * 128:(cc + 1) * 128, 128:512])
            for cc in range(8):
                nc.sync.dma_start(out=wv_t[cc][:], in_=wv[cc * 128:(cc + 1) * 128, :])
            dma_xT(1)
            dma_wqk_half(1)
            dma_xT(2)
            dma_xT(3)
            for cl in range(4):
                nc.sync.dma_start(out=wp_t[cl][:], in_=wp[cl * 128:(cl + 1) * 128, :])

            # persistent intermediates
            qk_t = []   # m 0..3 -> q^T chunks (scaled+biased), 4..7 -> k^T
            for m in range(8):
                qk_t.append(qkpool.tile([128, N], BF16, tag=f"qk{m}", name=f"qk{m}"))
            v_t = []    # [v | 1] per head: 8 groups of 65 cols
            for kc in range(16):
                v_t.append(vpool.tile([128, NH * 65], BF16, tag=f"v{kc}",
                                      name=f"v{kc}"))
            ao_t = []   # ao_t[p]: pair p normalized out^T (128 ch x N)
            for p in range(4):
                ao_t.append(aopool.tile([128, N], BF16, tag=f"ao{p}", name=f"ao{p}"))

            # ---------------- unit generators (one PE MM per step) --------
            # All unit PSUM goes through `aux` (allocated lazily at first
            # step so buffer rotation follows true issue order).
            # Unit epilogues (PSUM evacuation on ACT/DVE) are NOT emitted
            # with the final MM: they go into epi_q and are flushed at the
            # next iteration top, AFTER that iteration's exps are emitted.
            # Otherwise the epilogue sits in the strict-FIFO ACT/DVE queue
            # waiting for its spread-out MM group and head-of-line blocks
            # the exps, which stalls the PE on ss-bank reuse.
            aux_ref = [None]
            epi_q = []

            def flush_epi():
                while epi_q:
                    epi_q.pop(0)()

            def qk_gen(m, j):
                aux = aux_ref[0]
                js = slice(j * 512, (j + 1) * 512)
                ws = slice(POS[m] * 128, (POS[m] + 1) * 128)
                ps = aux.tile([128, 512], F32, tag="ps", name="ps")
                for cc in range(8):
                    nc.tensor.matmul(ps[:], wqk_t[cc][:, ws], xT_t[cc][:, js],
                                     start=(cc == 0), stop=(cc == 7))
                    if cc < 7:
                        yield

                def epi():
                    if (m + j) % 2 == 1:
                        nc.vector.tensor_scalar(
                            qk_t[m][:, js], ps[:], SCALE if m < 4 else 1.0,
                            bqk_t[:, m:m + 1], Mult, Add)
                    else:
                        nc.scalar.activation(
                            qk_t[m][:, js], ps[:], Ident,
                            bias=bqk_t[:, m:m + 1],
                            scale=SCALE if m < 4 else 1.0)
                epi_q.append(epi)
                yield

            def v_gen(kc):
                aux = aux_ref[0]
                ps = aux.tile([128, 512], F32, tag="ps", name="ps")
                for cc in range(8):
                    nc.tensor.matmul(ps[:], xT_t[cc][:, kc * 128:(kc + 1) * 128],
                                     wv_t[cc][:], start=(cc == 0), stop=(cc == 7))
                    if cc < 7:
                        yield

                def epi():
                    v3 = v_t[kc].rearrange("p (h e) -> p h e", h=NH)
                    nc.vector.memset(v3[:, :, 64:65], 1.0)
                    nc.vector.tensor_copy(
                        v3[:, :, 0:64],
                        ps.rearrange("p (h e) -> p h e", e=64))
                epi_q.append(epi)
                yield

            def proj_gen(m2, j, cls=(0, 1, 2, 3), out_dram=None, pool=None,
                         tag="ps"):
                aux = pool if pool is not None else aux_ref[0]
                js = slice(j * 512, (j + 1) * 512)
                py = aux.tile([128, 512], F32, tag=tag, name="py")
                for i, cl in enumerate(cls):
                    nc.tensor.matmul(py[:], wp_t[cl][:, m2 * 128:(m2 + 1) * 128],
                                     ao_t[cl][:, js], start=(i == 0),
                                     stop=(i == len(cls) - 1))
                    if i < len(cls) - 1:
                        yield

                def epi():
                    yt = ytpool.tile([128, 512], F32, tag="yt", name="yt")
                    # in-loop: ACT (DVE is the more loaded engine mid-loop);
                    # tail (yT2): alternate so the 8 copies split engines
                    if out_dram is not None and m2 % 2 == 0:
                        nc.vector.tensor_copy(yt[:], py[:])
                    else:
                        nc.scalar.activation(yt[:], py[:], Copy)
                    if out_dram is None:
                        nc.sync.dma_start(out=yT[m2 * 128:(m2 + 1) * 128, js],
                                          in_=yt[:])
                    else:
                        nc.sync.dma_start(out=out_dram[m2 * 128:(m2 + 1) * 128, :],
                                          in_=yt[:])
                epi_q.append(epi)
                yield

            # ---------------- filler machinery ----------------
            queue = []          # pending generators
            cur = [None]        # in-flight generator

            def push(*gens):
                queue.extend(gens)

            def fill(n):
                for _ in range(n):
                    while True:
                        if cur[0] is None:
                            if not queue:
                                return
                            cur[0] = queue.pop(0)
                        try:
                            next(cur[0])
                            break
                        except StopIteration:
                            cur[0] = None

            def run_unit(g):
                for _ in g:
                    pass

            # ---------------- attention helpers ----------------
            def sc_pair(ssEp, ssOp, p, kc, js):
                # serial full-mode score MMs (no tile_position): row-tile
                # pairing saved ~216ns/kc of overlap but cost ~105ns leader
                # wait + ~119ns tile<->full mode-switch penalty on the next
                # full-mode MM - a net loss.
                ssE = ssEp.tile([128, 512], F32, tag="ssE", name="ssE")
                ssO = ssOp.tile([128, 512], F32, tag="ssO", name="ssO")
                ks = slice(kc * 128, (kc + 1) * 128)
                nc.tensor.matmul(ssE[:], qk_t[4 + p][0:64, ks], qk_t[p][0:64, js],
                                 start=True, stop=True)
                nc.tensor.matmul(ssO[:], qk_t[4 + p][64:128, ks],
                                 qk_t[p][64:128, js], start=True, stop=True)
                return ssE, ssO

            def exp_pair(ssE, ssO, kc):
                ptE = ptpool.tile([128, 512], BF16, tag="ptE", name="ptE")
                ptO = ptpool.tile([128, 512], BF16, tag="ptO", name="ptO")
                nc.scalar.activation(ptE[:], ssE[:], Exp)
                if kc < 14:
                    nc.vector.tensor_scalar(
                        ptO.bitcast(I16)[:], ssO[:], A_EXP, B_EXP, Mult, Add)
                else:
                    nc.scalar.activation(ptO[:], ssO[:], Exp)
                return ptE, ptO

            def av_pair(avE, avO, p, kc, pts):
                ptE, ptO = pts[kc]
                he, ho = 2 * p, 2 * p + 1
                nc.tensor.matmul(avE[0:65, :], v_t[kc][:, he * 65:he * 65 + 65],
                                 ptE[:], start=(kc == 0), stop=(kc == 15))
                nc.tensor.matmul(avO[0:65, :], v_t[kc][:, ho * 65:ho * 65 + 65],
                                 ptO[:], start=(kc == 0), stop=(kc == 15))

            def norm_pre(av):
                # av rows 0:64 = out^T, row 64 = Z. Split from the mul so
                # the DVE mul (which waits on the GPSIMD broadcast) can be
                # emitted a batch later and never blocks the DVE queue.
                z1 = rpool.tile([1, 512], F32, tag="z1", name="z1")
                nc.vector.tensor_copy(z1[:], av[64:65, :])
                r1 = rpool.tile([1, 512], F32, tag="r1", name="r1")
                nc.vector.reciprocal_approx_fast(out=r1[:], in_=z1[:])
                rb = rpool.tile([64, 512], F32, tag="rb", name="rb")
                nc.gpsimd.partition_broadcast(rb[:], r1[:])
                return rb

            def norm_mul(av, p, po, js_, rb):
                nc.vector.tensor_mul(ao_t[p][po:po + 64, js_], av[0:64, :], rb[:])

            # ---------------- main ----------------
            with (
                tc.tile_pool(name="ssE", bufs=2, space="PSUM") as ssEp,
                tc.tile_pool(name="ssO", bufs=2, space="PSUM") as ssOp,
                tc.tile_pool(name="avE", bufs=1, space="PSUM") as avEp,
                tc.tile_pool(name="avO", bufs=1, space="PSUM") as avOp,
                tc.tile_pool(name="aux", bufs=2, space="PSUM") as aux,
            ):
                aux_ref[0] = aux

                # prologue: k(m4) j0/j1, q(m0) j0, v(0..3); k(m4) j2/j3 via queue
                for g in (qk_gen(4, 0), qk_gen(0, 0), v_gen(0), v_gen(1),
                          v_gen(2), v_gen(3), qk_gen(4, 1)):
                    run_unit(g)
                    flush_epi()

                push(qk_gen(4, 2), qk_gen(4, 3),
                     qk_gen(1, 0), qk_gen(5, 0), qk_gen(5, 1), qk_gen(5, 2),
                     qk_gen(5, 3), qk_gen(2, 0), qk_gen(6, 0), qk_gen(6, 1),
                     qk_gen(6, 2), qk_gen(6, 3), qk_gen(3, 0), qk_gen(7, 0),
                     qk_gen(7, 1), qk_gen(7, 2), qk_gen(7, 3),
                     qk_gen(0, 1), qk_gen(1, 1), qk_gen(2, 1), qk_gen(3, 1))

                pend = [None]
                for ji in range(4):
                    for p in range(4):
                        j = ji
                        js = slice(j * 512, (j + 1) * 512)
                        if (ji, p) == (1, 0):
                            push(*[proj_gen(m2, 0) for m2 in range(8)])
                            push(qk_gen(0, 2), qk_gen(1, 2), qk_gen(2, 2),
                                 qk_gen(3, 2))
                        elif (ji, p) == (2, 0):
                            push(*[proj_gen(m2, 1) for m2 in range(8)])
                            push(qk_gen(0, 3), qk_gen(1, 3), qk_gen(2, 3),
                                 qk_gen(3, 3))
                        elif (ji, p) == (3, 0):
                            push(*[proj_gen(m2, 2) for m2 in range(8)])

                        # kc processed in batches of 2: both score pairs run
                        # back-to-back in row-tile mode (one tile<->full mode
                        # switch per batch instead of per kc; each switch
                        # costs the first following MM ~105-120ns).
                        pts = {}
                        avE = avO = None
                        for kc2 in range(8):
                            a, b = 2 * kc2, 2 * kc2 + 1
                            # flush BEFORE the ss MMs: emission order defines
                            # dependency tracking, so any epilogue whose
                            # output the ss (or other) MMs read must be
                            # emitted first. Units always finish >=1 fill
                            # slot before the flush, so the epilogue's wait
                            # is (near-)resolved and doesn't HOL-block exps.
                            flush_epi()
                            ssA = sc_pair(ssEp, ssOp, p, a, js)
                            pts[a] = exp_pair(*ssA, a)
                            ssB = sc_pair(ssEp, ssOp, p, b, js)
                            pts[b] = exp_pair(*ssB, b)
                            if pend[0] is not None and kc2 == 0:
                                pavE, pavO, pp, pjs, ppts = pend[0]
                                av_pair(pavE, pavO, pp, 14, ppts)
                                av_pair(pavE, pavO, pp, 15, ppts)
                                rbE = norm_pre(pavE)
                                rbO = norm_pre(pavO)
                                pend[0] = (pavE, pavO, pp, pjs, rbE, rbO)
                            if kc2 == 1:
                                if pend[0] is not None:
                                    pavE, pavO, pp, pjs, rbE, rbO = pend[0]
                                    norm_mul(pavE, pp, 0, pjs, rbE)
                                    norm_mul(pavO, pp, 64, pjs, rbO)
                                    pend[0] = None
                                avE = avEp.tile([128, 512], F32, tag="avE",
                                                name="avE")
                                avO = avOp.tile([128, 512], F32, tag="avO",
                                                name="avO")
                            if kc2 >= 1:
                                av_pair(avE, avO, p, a - 2, pts)
                                av_pair(avE, avO, p, b - 2, pts)
                            # fillers: spread evenly, heavier early (qkv gen)
                            if (ji, p) == (0, 0):
                                if kc2 < 6:
                                    run_unit(v_gen(a + 4))
                                    run_unit(v_gen(b + 4))
                                    fill(4)
                                else:
                                    fill(6)
                            elif ji == 0:
                                fill(6)
                            elif (ji, p) == (3, 3):
                                if kc2 == 1:
                                    push(*[proj_gen(m2, 3, cls=(0, 1, 2))
                                           for m2 in range(8)])
                                fill(4)
                            else:
                                fill(2)
                        pend[0] = (avE, avO, p, js, pts)

                # ---------------- tail ----------------
                pavE, pavO, pp, pjs, ppts = pend[0]
                av_pair(pavE, pavO, pp, 14, ppts)
                av_pair(pavE, pavO, pp, 15, ppts)
                rbE = norm_pre(pavE)
                rbO = norm_pre(pavO)
                norm_mul(pavE, pp, 0, pjs, rbE)
                norm_mul(pavO, pp, 64, pjs, rbO)
                pend[0] = None
                fill(64)   # drain any queue leftovers
                flush_epi()
                # keep-warm dummies while the final norm chain runs
                dum = aux.tile([128, 512], F32, tag="ps", name="dum")
                for _ in range(22):
                    nc.tensor.matmul(dum[:], wp_t[0][:, 0:128], ao_t[0][:, 0:512],
                                     start=True, stop=True)
                # final 8 single-MM proj units (cl=3 only) -> yT2. Cycle
                # their PSUM through the now-free av banks too (4-deep
                # rotation): with aux's 2 buffers alone, unit m2+2's MM
                # stalls on unit m2's epilogue copy.
                tail_pools = [(avEp, "avE"), (avOp, "avO"), (aux, "ps"),
                              (aux, "ps")]
                for m2 in range(8):
                    tp_, tg_ = tail_pools[m2 % 4]
                    run_unit(proj_gen(m2, 3, cls=(3,), out_dram=yT2,
                                      pool=tp_, tag=tg_))
                    flush_epi()

    nc.compile()
    return nc


def make_in_maps(x, w_qkv, b_qkv, w_proj):
    np_bf = mybir.dt.np(BF16)
    x = np.asarray(x, np.float32)
    w_qkv = np.asarray(w_qkv, np.float32)
    b_qkv = np.asarray(b_qkv, np.float32)
    w_proj = np.asarray(w_proj, np.float32)
    in_maps = []
    for c in range(NCORES):
        b, g = divmod(c, 2)
        h0 = g * NH
        qs = slice(h0 * D, h0 * D + CL)
        ks = slice(C + h0 * D, C + h0 * D + CL)
        vs = slice(2 * C + h0 * D, 2 * C + h0 * D + CL)
        wqk_m = np.concatenate([w_qkv[:, qs], w_qkv[:, ks]], axis=1)
        # permute 128-col blocks into consumer priority order
        wqk_m = np.concatenate([wqk_m[:, m * 128:(m + 1) * 128] for m in PR],
                               axis=1)
        bq = b_qkv[qs] * SCALE
        bk = b_qkv[ks]
        bqk_m = np.concatenate([bq, bk]).reshape(8, 128).T  # [128, 8] col-chunks
        in_maps.append({
            "xT": np.ascontiguousarray(x[b].T).astype(np_bf),
            "wqk": np.ascontiguousarray(wqk_m).astype(np_bf),
            "wv": np.ascontiguousarray(w_qkv[:, vs]).astype(np_bf),
            "wp": np.ascontiguousarray(w_proj[h0 * D:h0 * D + CL, :]).astype(np_bf),
            "bqk": np.ascontiguousarray(bqk_m, np.float32),
        })
    return in_maps


def run(x, w_qkv, b_qkv, w_proj, b_proj, mm_dt=MM_DT, **spmd_kwargs):
    if "nc" not in _CACHE:
        _CACHE["nc"] = build_nc()
    nc = _CACHE["nc"]
    in_maps = make_in_maps(x, w_qkv, b_qkv, w_proj)
    res = run_bass_kernel_spmd(nc, in_maps, core_ids=list(range(NCORES)),
                               **spmd_kwargs)
    # v-bias passes through softmax averaging exactly (weights sum to 1),
    # so its projected contribution folds into the output bias on the host.
    b_eff = (np.asarray(b_proj, np.float64)
             + np.asarray(b_qkv, np.float64)[2 * C:] @ np.asarray(w_proj, np.float64)
             ).astype(np.float32)
    out = np.empty((B, N, C), np.float32)
    for b in range(B):
        acc = res.results[2 * b]["yT"] + res.results[2 * b + 1]["yT"]
        acc[:, 3 * 512:] += res.results[2 * b]["yT2"] + res.results[2 * b + 1]["yT2"]
        out[b] = acc.T + b_eff[None, :]
    return out, res


def kernel(x, w_qkv, b_qkv, w_proj, b_proj):
    out, _ = run(x, w_qkv, b_qkv, w_proj, b_proj)
    return out
